# revision 23
# baseline (speedup 1.0000x reference)
"""Trainium2 kernel for nn_AttackModule (retrieval_knn).

Contract: kernel(**inputs) takes FULL inputs
    utterance [32, 64] int, emb_table [50257, 512] f32, data_grad [32, 64, 512] f32
and returns the FULL output tuple (nn_idx [32, 64] int32, sim [32, 64, 50257] f32),
matching reference.reference().

Strategy (vocab-sharded tensor parallel over 8 NeuronCores):
 - Host (cheap, O(MB)): embedding gather, FGSM perturbation, norm scaling.
 - Device (the heavy part, O(400MB) output): sim = (perturbed/|p|) @ (emb/|t|)^T,
   vocab axis sharded 8 ways (6400 padded columns per core), fp32r matmuls.
 - Host: assemble shards, argmax over vocab.
"""

import numpy as np

B, S, E, V = 32, 64, 512, 50257
EPS = 0.4
TOP_K_FRAC = 0.5
STD_MULT = 3.0
COS_EPS = 1e-8

M = B * S            # 2048 tokens
P = 128              # partitions
KC = E // P          # 4 contraction chunks
NTILE = 512          # matmul moving free dim
VS = 6400            # vocab shard per core (12 n-tiles of 512 + 1 of 256)
NTF = 12             # full n-tiles
NTAIL = 256          # tail n-tile width
MT = M // P          # 16
NCORES = 8
VPAD = VS * NCORES   # 51200

_CACHE = {}


# tunables (module-level so a bench harness can override before first build)
MG = 4               # m-tiles per output DMA batch
ET_AHEAD = 6         # et tiles of prefetch depth
OB_BUFS = 8          # output staging strips
PS_BUFS = 8          # psum banks in rotation


def _build_nc():
    import concourse.bacc as bacc
    import concourse.mybir as mybir
    import concourse.tile as tile

    f32 = mybir.dt.float32
    f32r = mybir.dt.float32r

    MB = 256                 # pt block width (m columns per block)
    NPT = M // MB            # 8 pt blocks

    nc = bacc.Bacc("TRN2")
    # host-swizzled: ptb[j, p, k, mm] = ptw[k*128+p, j*256+mm]
    ptb = nc.dram_tensor("ptb", [NPT, P, KC, MB], f32r, kind="ExternalInput")
    # host-swizzled: etb[n, p, k, nn] = ets[k*128+p, n*512+nn] (full tiles)
    etb = nc.dram_tensor("etb", [NTF, P, KC, NTILE], f32r, kind="ExternalInput")
    # tail tile: etb_t[p, k, nn] = ets[k*128+p, NTF*512+nn]
    etb_t = nc.dram_tensor("etb_t", [P, KC, NTAIL], f32r, kind="ExternalInput")
    sim = nc.dram_tensor("sim", [M, VS], f32, kind="ExternalOutput")

    with tile.TileContext(nc) as tc:
        with tc.tile_pool(name="pt_pool", bufs=1) as pt_pool, \
             tc.tile_pool(name="et_pool", bufs=ET_AHEAD) as et_pool, \
             tc.tile_pool(name="et_t_pool", bufs=1) as et_t_pool, \
             tc.tile_pool(name="ob_pool", bufs=OB_BUFS) as ob_pool, \
             tc.tile_pool(name="ob_t_pool", bufs=2) as ob_t_pool, \
             tc.tile_pool(name="ps_pool", bufs=PS_BUFS, space="PSUM") as ps_pool:

            et_sbs = {}

            def load_et(n):
                et_sbs[n] = et_pool.tile([P, KC, NTILE], f32r, tag="et",
                                         name=f"et_{n}")
                nc.sync.dma_start(et_sbs[n][:], etb[n])

            # FIFO order on the input ring: pt0, et0, pt1-7, et1-5
            pt_sb = [pt_pool.tile([P, KC, MB], f32r, tag=f"pt_{j}", name=f"pt_{j}")
                     for j in range(NPT)]
            nc.sync.dma_start(pt_sb[0][:], ptb[0])
            load_et(0)
            for j in range(1, NPT):
                nc.sync.dma_start(pt_sb[j][:], ptb[j])
            for n in range(1, ET_AHEAD):
                load_et(n)
            et_t = et_t_pool.tile([P, KC, NTAIL], f32r, tag="et_t", name="et_t")

            def do_tile(col0, et_sb, width, ob_pool_, tagw):
                for mg in range(MT // MG):
                    ob = ob_pool_.tile([P, MG, width], f32, tag=tagw,
                                       name=f"ob_{col0}_{mg}")
                    for mi in range(MG):
                        m = mg * MG + mi
                        ps = ps_pool.tile([P, NTILE], f32, tag="ps",
                                          name=f"ps_{col0}_{m}")[:, :width]
                        for k in range(KC):
                            nc.tensor.matmul(
                                ps[:],
                                pt_sb[m // 2][:, k, (m % 2) * P:(m % 2) * P + P],
                                et_sb[:, k, :],
                                start=(k == 0),
                                stop=(k == KC - 1),
                            )
                        nc.scalar.copy(ob[:, mi, :], ps[:])
                    nc.scalar.dma_start(
                        sim[mg * MG * P:(mg + 1) * MG * P, col0:col0 + width]
                        .rearrange("(j p) n -> p j n", p=P),
                        ob[:])

            for n in range(NTF):
                if n + ET_AHEAD < NTF:
                    load_et(n + ET_AHEAD)
                elif n + ET_AHEAD == NTF:
                    nc.sync.dma_start(et_t[:], etb_t[:])
                do_tile(n * NTILE, et_sbs[n], NTILE, ob_pool, "ob")
            do_tile(NTF * NTILE, et_t, NTAIL, ob_t_pool, "obt")
    nc.compile()
    return nc


def _get_nc():
    if "nc" not in _CACHE:
        _CACHE["nc"] = _build_nc()
    return _CACHE["nc"]


def _fgsm_perturb(in_data, data_grad):
    """Replicates reference.fgsm_attack in numpy float32."""
    g = data_grad.astype(np.float32)
    abs_grad = np.abs(g).sum(axis=-1)                      # [B, S]
    k = int(TOP_K_FRAC * S)
    top_idx = np.argsort(-abs_grad, axis=1, kind="stable")[:, :k]
    tok_mask = np.zeros((B, S), dtype=bool)
    tok_mask[np.arange(B)[:, None], top_idx] = True
    mask = np.broadcast_to(tok_mask[:, :, None], g.shape)
    gm = np.where(mask, g, np.float32(0.0))
    n = B * k * E
    mean = np.float32(gm.sum(dtype=np.float32) / np.float32(n))
    dev = np.where(mask, g - mean, np.float32(0.0)).astype(np.float32)
    var = np.float32((dev * dev).sum(dtype=np.float32) / np.float32(n - 1))
    std = np.sqrt(var)
    lb = mean - std * np.float32(STD_MULT)
    ub = mean + std * np.float32(STD_MULT)
    outside = (g < lb) | (g > ub)
    g2 = np.where(mask & outside, g, np.float32(0.0))
    return (in_data + np.float32(EPS) * np.sign(g2)).astype(np.float32)


def prepare_in_maps(utterance, emb_table, data_grad):
    utterance = np.asarray(utterance)
    emb_table = np.asarray(emb_table, dtype=np.float32)
    data_grad = np.asarray(data_grad, dtype=np.float32)

    # ---- host: gather + FGSM + norms ----
    in_data = emb_table[utterance]                         # [B, S, E]
    perturbed = _fgsm_perturb(in_data, data_grad)          # [B, S, E]
    pflat = perturbed.reshape(M, E)

    p_norm = np.sqrt((pflat * pflat).sum(axis=1, dtype=np.float32))
    p_norm = np.maximum(p_norm, np.float32(COS_EPS))
    t_norm = np.sqrt((emb_table * emb_table).sum(axis=1, dtype=np.float32))
    t_norm = np.maximum(t_norm, np.float32(COS_EPS))

    ptw = np.ascontiguousarray((pflat / p_norm[:, None]).T, dtype=np.float32)
    # swizzle: ptb[j, p, k, mm] = ptw[k*128+p, j*256+mm]
    ptb = np.ascontiguousarray(
        ptw.reshape(KC, P, M // 256, 256).transpose(2, 1, 0, 3))

    ets_full = (emb_table / t_norm[:, None]).astype(np.float32)
    ets_pad = np.zeros((VPAD, E), dtype=np.float32)
    ets_pad[:V] = ets_full

    in_maps = []
    for c in range(NCORES):
        shard = np.ascontiguousarray(
            ets_pad[c * VS:(c + 1) * VS].T, dtype=np.float32)   # [E, VS]
        # swizzle: etb[n, p, k, nn] = shard[k*128+p, n*512+nn]
        full = shard[:, :NTF * NTILE]
        etb = np.ascontiguousarray(
            full.reshape(KC, P, NTF, NTILE).transpose(2, 1, 0, 3))
        tail = shard[:, NTF * NTILE:]
        etb_t = np.ascontiguousarray(
            tail.reshape(KC, P, NTAIL).transpose(1, 0, 2))
        in_maps.append({"ptb": ptb, "etb": etb, "etb_t": etb_t})
    return in_maps


def run_device(in_maps, **kwargs):
    from concourse.bass_utils import run_bass_kernel_spmd

    nc = _get_nc()
    return run_bass_kernel_spmd(nc, in_maps, core_ids=list(range(NCORES)), **kwargs)


def postprocess(res):
    sim_out = np.empty((B, S, V), dtype=np.float32)
    sim_flat = sim_out.reshape(M, V)
    for c in range(NCORES):
        lo = c * VS
        hi = min(lo + VS, V)
        if hi <= lo:
            break
        sim_flat[:, lo:hi] = res.results[c]["sim"][:, :hi - lo]

    nn_idx = np.argmax(sim_flat, axis=1).astype(np.int32).reshape(B, S)
    return nn_idx, sim_out


def kernel(utterance, emb_table, data_grad):
    in_maps = prepare_in_maps(utterance, emb_table, data_grad)
    res = run_device(in_maps)
    return postprocess(res)


# revision 24
# speedup vs baseline: 1.0713x; 1.0713x over previous
"""Trainium2 kernel for nn_AttackModule (retrieval_knn).

Contract: kernel(**inputs) takes FULL inputs
    utterance [32, 64] int, emb_table [50257, 512] f32, data_grad [32, 64, 512] f32
and returns the FULL output tuple (nn_idx [32, 64] int32, sim [32, 64, 50257] f32),
matching reference.reference().

Strategy (vocab-sharded tensor parallel over 8 NeuronCores):
 - Host (cheap, O(MB)): embedding gather, FGSM perturbation, norm scaling.
 - Device (the heavy part, O(400MB) output): sim = (perturbed/|p|) @ (emb/|t|)^T,
   vocab axis sharded 8 ways (6400 padded columns per core), fp32r matmuls.
 - Host: assemble shards, argmax over vocab.
"""

import numpy as np

B, S, E, V = 32, 64, 512, 50257
EPS = 0.4
TOP_K_FRAC = 0.5
STD_MULT = 3.0
COS_EPS = 1e-8

M = B * S            # 2048 tokens
P = 128              # partitions
KC = E // P          # 4 contraction chunks
NTILE = 512          # matmul moving free dim
VS = 6400            # vocab shard per core (12 n-tiles of 512 + 1 of 256)
NTF = 12             # full n-tiles
NTAIL = 256          # tail n-tile width
MT = M // P          # 16
NCORES = 8
VPAD = VS * NCORES   # 51200

_CACHE = {}


# tunables (module-level so a bench harness can override before first build)
USE_BF16 = 1         # bf16 matmul operands (halves input DMA; psum/output stay f32)
MG = 4               # m-tiles per output DMA batch
ET_AHEAD = 6         # et tiles of prefetch depth
OB_BUFS = 8          # output staging strips
PS_BUFS = 8          # psum banks in rotation


def _build_nc():
    import concourse.bacc as bacc
    import concourse.mybir as mybir
    import concourse.tile as tile

    f32 = mybir.dt.float32
    f32r = mybir.dt.bfloat16 if USE_BF16 else mybir.dt.float32r

    MB = 256                 # pt block width (m columns per block)
    NPT = M // MB            # 8 pt blocks

    nc = bacc.Bacc("TRN2")
    # host-swizzled: ptb[j, p, k, mm] = ptw[k*128+p, j*256+mm]
    ptb = nc.dram_tensor("ptb", [NPT, P, KC, MB], f32r, kind="ExternalInput")
    # host-swizzled: etb[n, p, k, nn] = ets[k*128+p, n*512+nn] (full tiles)
    etb = nc.dram_tensor("etb", [NTF, P, KC, NTILE], f32r, kind="ExternalInput")
    # tail tile: etb_t[p, k, nn] = ets[k*128+p, NTF*512+nn]
    etb_t = nc.dram_tensor("etb_t", [P, KC, NTAIL], f32r, kind="ExternalInput")
    sim = nc.dram_tensor("sim", [M, VS], f32, kind="ExternalOutput")

    with tile.TileContext(nc) as tc:
        with tc.tile_pool(name="pt_pool", bufs=1) as pt_pool, \
             tc.tile_pool(name="et_pool", bufs=ET_AHEAD) as et_pool, \
             tc.tile_pool(name="et_t_pool", bufs=1) as et_t_pool, \
             tc.tile_pool(name="ob_pool", bufs=OB_BUFS) as ob_pool, \
             tc.tile_pool(name="ob_t_pool", bufs=2) as ob_t_pool, \
             tc.tile_pool(name="ps_pool", bufs=PS_BUFS, space="PSUM") as ps_pool:

            et_sbs = {}

            def load_et(n):
                et_sbs[n] = et_pool.tile([P, KC, NTILE], f32r, tag="et",
                                         name=f"et_{n}")
                nc.sync.dma_start(et_sbs[n][:], etb[n])

            # FIFO order on the input ring: pt0, et0, pt1-7, et1-5
            pt_sb = [pt_pool.tile([P, KC, MB], f32r, tag=f"pt_{j}", name=f"pt_{j}")
                     for j in range(NPT)]
            nc.sync.dma_start(pt_sb[0][:], ptb[0])
            load_et(0)
            for j in range(1, NPT):
                nc.sync.dma_start(pt_sb[j][:], ptb[j])
            for n in range(1, ET_AHEAD):
                load_et(n)
            et_t = et_t_pool.tile([P, KC, NTAIL], f32r, tag="et_t", name="et_t")

            def do_tile(col0, et_sb, width, ob_pool_, tagw):
                for mg in range(MT // MG):
                    ob = ob_pool_.tile([P, MG, width], f32, tag=tagw,
                                       name=f"ob_{col0}_{mg}")
                    for mi in range(MG):
                        m = mg * MG + mi
                        ps = ps_pool.tile([P, NTILE], f32, tag="ps",
                                          name=f"ps_{col0}_{m}")[:, :width]
                        for k in range(KC):
                            nc.tensor.matmul(
                                ps[:],
                                pt_sb[m // 2][:, k, (m % 2) * P:(m % 2) * P + P],
                                et_sb[:, k, :],
                                start=(k == 0),
                                stop=(k == KC - 1),
                            )
                        nc.scalar.copy(ob[:, mi, :], ps[:])
                    nc.scalar.dma_start(
                        sim[mg * MG * P:(mg + 1) * MG * P, col0:col0 + width]
                        .rearrange("(j p) n -> p j n", p=P),
                        ob[:])

            for n in range(NTF):
                if n + ET_AHEAD < NTF:
                    load_et(n + ET_AHEAD)
                elif n + ET_AHEAD == NTF:
                    nc.sync.dma_start(et_t[:], etb_t[:])
                do_tile(n * NTILE, et_sbs[n], NTILE, ob_pool, "ob")
            do_tile(NTF * NTILE, et_t, NTAIL, ob_t_pool, "obt")
    nc.compile()
    return nc


def _get_nc():
    if "nc" not in _CACHE:
        _CACHE["nc"] = _build_nc()
    return _CACHE["nc"]


def _fgsm_perturb(in_data, data_grad):
    """Replicates reference.fgsm_attack in numpy float32."""
    g = data_grad.astype(np.float32)
    abs_grad = np.abs(g).sum(axis=-1)                      # [B, S]
    k = int(TOP_K_FRAC * S)
    top_idx = np.argsort(-abs_grad, axis=1, kind="stable")[:, :k]
    tok_mask = np.zeros((B, S), dtype=bool)
    tok_mask[np.arange(B)[:, None], top_idx] = True
    mask = np.broadcast_to(tok_mask[:, :, None], g.shape)
    gm = np.where(mask, g, np.float32(0.0))
    n = B * k * E
    mean = np.float32(gm.sum(dtype=np.float32) / np.float32(n))
    dev = np.where(mask, g - mean, np.float32(0.0)).astype(np.float32)
    var = np.float32((dev * dev).sum(dtype=np.float32) / np.float32(n - 1))
    std = np.sqrt(var)
    lb = mean - std * np.float32(STD_MULT)
    ub = mean + std * np.float32(STD_MULT)
    outside = (g < lb) | (g > ub)
    g2 = np.where(mask & outside, g, np.float32(0.0))
    return (in_data + np.float32(EPS) * np.sign(g2)).astype(np.float32)


def prepare_in_maps(utterance, emb_table, data_grad):
    utterance = np.asarray(utterance)
    emb_table = np.asarray(emb_table, dtype=np.float32)
    data_grad = np.asarray(data_grad, dtype=np.float32)

    # ---- host: gather + FGSM + norms ----
    in_data = emb_table[utterance]                         # [B, S, E]
    perturbed = _fgsm_perturb(in_data, data_grad)          # [B, S, E]
    pflat = perturbed.reshape(M, E)

    p_norm = np.sqrt((pflat * pflat).sum(axis=1, dtype=np.float32))
    p_norm = np.maximum(p_norm, np.float32(COS_EPS))
    t_norm = np.sqrt((emb_table * emb_table).sum(axis=1, dtype=np.float32))
    t_norm = np.maximum(t_norm, np.float32(COS_EPS))

    ptw = np.ascontiguousarray((pflat / p_norm[:, None]).T, dtype=np.float32)
    # swizzle: ptb[j, p, k, mm] = ptw[k*128+p, j*256+mm]
    ptb = np.ascontiguousarray(
        ptw.reshape(KC, P, M // 256, 256).transpose(2, 1, 0, 3))

    ets_full = (emb_table / t_norm[:, None]).astype(np.float32)
    ets_pad = np.zeros((VPAD, E), dtype=np.float32)
    ets_pad[:V] = ets_full

    in_maps = []
    for c in range(NCORES):
        shard = np.ascontiguousarray(
            ets_pad[c * VS:(c + 1) * VS].T, dtype=np.float32)   # [E, VS]
        # swizzle: etb[n, p, k, nn] = shard[k*128+p, n*512+nn]
        full = shard[:, :NTF * NTILE]
        etb = np.ascontiguousarray(
            full.reshape(KC, P, NTF, NTILE).transpose(2, 1, 0, 3))
        tail = shard[:, NTF * NTILE:]
        etb_t = np.ascontiguousarray(
            tail.reshape(KC, P, NTAIL).transpose(1, 0, 2))
        in_maps.append({"ptb": ptb, "etb": etb, "etb_t": etb_t})
    if USE_BF16:
        import ml_dtypes
        bf = ml_dtypes.bfloat16
        cache = {}
        for im in in_maps:
            for k2 in im:
                key = id(im[k2])
                if key not in cache:
                    cache[key] = im[k2].astype(bf)
                im[k2] = cache[key]
    return in_maps


def run_device(in_maps, **kwargs):
    from concourse.bass_utils import run_bass_kernel_spmd

    nc = _get_nc()
    return run_bass_kernel_spmd(nc, in_maps, core_ids=list(range(NCORES)), **kwargs)


def postprocess(res):
    sim_out = np.empty((B, S, V), dtype=np.float32)
    sim_flat = sim_out.reshape(M, V)
    for c in range(NCORES):
        lo = c * VS
        hi = min(lo + VS, V)
        if hi <= lo:
            break
        sim_flat[:, lo:hi] = res.results[c]["sim"][:, :hi - lo]

    nn_idx = np.argmax(sim_flat, axis=1).astype(np.int32).reshape(B, S)
    return nn_idx, sim_out


def kernel(utterance, emb_table, data_grad):
    in_maps = prepare_in_maps(utterance, emb_table, data_grad)
    res = run_device(in_maps)
    return postprocess(res)


# revision 26
# speedup vs baseline: 1.0843x; 1.0121x over previous
"""Trainium2 kernel for nn_AttackModule (retrieval_knn).

Contract: kernel(**inputs) takes FULL inputs
    utterance [32, 64] int, emb_table [50257, 512] f32, data_grad [32, 64, 512] f32
and returns the FULL output tuple (nn_idx [32, 64] int32, sim [32, 64, 50257] f32),
matching reference.reference().

Strategy (vocab-sharded tensor parallel over 8 NeuronCores):
 - Host (cheap, O(MB)): embedding gather, FGSM perturbation, norm scaling.
 - Device (the heavy part, O(400MB) output): sim = (perturbed/|p|) @ (emb/|t|)^T,
   vocab axis sharded 8 ways (6400 padded columns per core), fp32r matmuls.
 - Host: assemble shards, argmax over vocab.
"""

import numpy as np

B, S, E, V = 32, 64, 512, 50257
EPS = 0.4
TOP_K_FRAC = 0.5
STD_MULT = 3.0
COS_EPS = 1e-8

M = B * S            # 2048 tokens
P = 128              # partitions
KC = E // P          # 4 contraction chunks
NTILE = 512          # matmul moving free dim
VS = 6400            # vocab shard per core (12 n-tiles of 512 + 1 of 256)
NTF = 12             # full n-tiles
NTAIL = 256          # tail n-tile width
MT = M // P          # 16
NCORES = 8
VPAD = VS * NCORES   # 51200

_CACHE = {}


# tunables (module-level so a bench harness can override before first build)
USE_BF16 = 1         # bf16 matmul operands (halves input DMA; psum/output stay f32)
OUT_SPLIT = 0        # alternate output DMAs between ACT-HWDGE and GpSimd-SWDGE rings
MG = 4               # m-tiles per output DMA batch
ET_AHEAD = 6         # et tiles of prefetch depth
OB_BUFS = 8          # output staging strips
PS_BUFS = 8          # psum banks in rotation


def _build_nc():
    import concourse.bacc as bacc
    import concourse.mybir as mybir
    import concourse.tile as tile

    f32 = mybir.dt.float32
    f32r = mybir.dt.bfloat16 if USE_BF16 else mybir.dt.float32r

    MB = 256                 # pt block width (m columns per block)
    NPT = M // MB            # 8 pt blocks

    nc = bacc.Bacc("TRN2")
    # host-swizzled: ptb[j, p, k, mm] = ptw[k*128+p, j*256+mm]
    ptb = nc.dram_tensor("ptb", [NPT, P, KC, MB], f32r, kind="ExternalInput")
    # host-swizzled: etb[n, p, k, nn] = ets[k*128+p, n*512+nn] (full tiles)
    etb = nc.dram_tensor("etb", [NTF, P, KC, NTILE], f32r, kind="ExternalInput")
    # tail tile: etb_t[p, k, nn] = ets[k*128+p, NTF*512+nn]
    etb_t = nc.dram_tensor("etb_t", [P, KC, NTAIL], f32r, kind="ExternalInput")
    sim = nc.dram_tensor("sim", [M, VS], f32, kind="ExternalOutput")

    with tile.TileContext(nc) as tc:
        with tc.tile_pool(name="pt_pool", bufs=1) as pt_pool, \
             tc.tile_pool(name="et_pool", bufs=ET_AHEAD) as et_pool, \
             tc.tile_pool(name="et_t_pool", bufs=1) as et_t_pool, \
             tc.tile_pool(name="ob_pool", bufs=OB_BUFS) as ob_pool, \
             tc.tile_pool(name="ob_t_pool", bufs=2) as ob_t_pool, \
             tc.tile_pool(name="ps_pool", bufs=PS_BUFS, space="PSUM") as ps_pool:

            et_sbs = {}

            def load_et(n):
                et_sbs[n] = et_pool.tile([P, KC, NTILE], f32r, tag="et",
                                         name=f"et_{n}")
                nc.sync.dma_start(et_sbs[n][:], etb[n])

            # FIFO order on the input ring: pt0, et0, pt1-7, et1-5
            pt_sb = [pt_pool.tile([P, KC, MB], f32r, tag=f"pt_{j}", name=f"pt_{j}")
                     for j in range(NPT)]
            nc.sync.dma_start(pt_sb[0][:], ptb[0])
            load_et(0)
            for j in range(1, NPT):
                nc.sync.dma_start(pt_sb[j][:], ptb[j])
            for n in range(1, ET_AHEAD):
                load_et(n)
            et_t = et_t_pool.tile([P, KC, NTAIL], f32r, tag="et_t", name="et_t")

            def do_tile(col0, et_sb, width, ob_pool_, tagw):
                for mg in range(MT // MG):
                    ob = ob_pool_.tile([P, MG, width], f32, tag=tagw,
                                       name=f"ob_{col0}_{mg}")
                    for mi in range(MG):
                        m = mg * MG + mi
                        ps = ps_pool.tile([P, NTILE], f32, tag="ps",
                                          name=f"ps_{col0}_{m}")[:, :width]
                        for k in range(KC):
                            nc.tensor.matmul(
                                ps[:],
                                pt_sb[m // 2][:, k, (m % 2) * P:(m % 2) * P + P],
                                et_sb[:, k, :],
                                start=(k == 0),
                                stop=(k == KC - 1),
                            )
                        nc.scalar.copy(ob[:, mi, :], ps[:])
                    eng = nc.gpsimd if (OUT_SPLIT and mg % 2 == 1) else nc.scalar
                    eng.dma_start(
                        sim[mg * MG * P:(mg + 1) * MG * P, col0:col0 + width]
                        .rearrange("(j p) n -> p j n", p=P),
                        ob[:])

            for n in range(NTF):
                if n + ET_AHEAD < NTF:
                    load_et(n + ET_AHEAD)
                elif n + ET_AHEAD == NTF:
                    nc.sync.dma_start(et_t[:], etb_t[:])
                do_tile(n * NTILE, et_sbs[n], NTILE, ob_pool, "ob")
            do_tile(NTF * NTILE, et_t, NTAIL, ob_t_pool, "obt")
    nc.compile()
    return nc


def _get_nc():
    if "nc" not in _CACHE:
        _CACHE["nc"] = _build_nc()
    return _CACHE["nc"]


def _fgsm_perturb(in_data, data_grad):
    """Replicates reference.fgsm_attack in numpy float32."""
    g = data_grad.astype(np.float32)
    abs_grad = np.abs(g).sum(axis=-1)                      # [B, S]
    k = int(TOP_K_FRAC * S)
    top_idx = np.argsort(-abs_grad, axis=1, kind="stable")[:, :k]
    tok_mask = np.zeros((B, S), dtype=bool)
    tok_mask[np.arange(B)[:, None], top_idx] = True
    mask = np.broadcast_to(tok_mask[:, :, None], g.shape)
    gm = np.where(mask, g, np.float32(0.0))
    n = B * k * E
    mean = np.float32(gm.sum(dtype=np.float32) / np.float32(n))
    dev = np.where(mask, g - mean, np.float32(0.0)).astype(np.float32)
    var = np.float32((dev * dev).sum(dtype=np.float32) / np.float32(n - 1))
    std = np.sqrt(var)
    lb = mean - std * np.float32(STD_MULT)
    ub = mean + std * np.float32(STD_MULT)
    outside = (g < lb) | (g > ub)
    g2 = np.where(mask & outside, g, np.float32(0.0))
    return (in_data + np.float32(EPS) * np.sign(g2)).astype(np.float32)


def prepare_in_maps(utterance, emb_table, data_grad):
    utterance = np.asarray(utterance)
    emb_table = np.asarray(emb_table, dtype=np.float32)
    data_grad = np.asarray(data_grad, dtype=np.float32)

    # ---- host: gather + FGSM + norms ----
    in_data = emb_table[utterance]                         # [B, S, E]
    perturbed = _fgsm_perturb(in_data, data_grad)          # [B, S, E]
    pflat = perturbed.reshape(M, E)

    p_norm = np.sqrt((pflat * pflat).sum(axis=1, dtype=np.float32))
    p_norm = np.maximum(p_norm, np.float32(COS_EPS))
    t_norm = np.sqrt((emb_table * emb_table).sum(axis=1, dtype=np.float32))
    t_norm = np.maximum(t_norm, np.float32(COS_EPS))

    ptw = np.ascontiguousarray((pflat / p_norm[:, None]).T, dtype=np.float32)
    # swizzle: ptb[j, p, k, mm] = ptw[k*128+p, j*256+mm]
    ptb = np.ascontiguousarray(
        ptw.reshape(KC, P, M // 256, 256).transpose(2, 1, 0, 3))

    ets_full = (emb_table / t_norm[:, None]).astype(np.float32)
    ets_pad = np.zeros((VPAD, E), dtype=np.float32)
    ets_pad[:V] = ets_full

    in_maps = []
    for c in range(NCORES):
        shard = np.ascontiguousarray(
            ets_pad[c * VS:(c + 1) * VS].T, dtype=np.float32)   # [E, VS]
        # swizzle: etb[n, p, k, nn] = shard[k*128+p, n*512+nn]
        full = shard[:, :NTF * NTILE]
        etb = np.ascontiguousarray(
            full.reshape(KC, P, NTF, NTILE).transpose(2, 1, 0, 3))
        tail = shard[:, NTF * NTILE:]
        etb_t = np.ascontiguousarray(
            tail.reshape(KC, P, NTAIL).transpose(1, 0, 2))
        in_maps.append({"ptb": ptb, "etb": etb, "etb_t": etb_t})
    if USE_BF16:
        import ml_dtypes
        bf = ml_dtypes.bfloat16
        cache = {}
        for im in in_maps:
            for k2 in im:
                key = id(im[k2])
                if key not in cache:
                    cache[key] = im[k2].astype(bf)
                im[k2] = cache[key]
    return in_maps


def run_device(in_maps, **kwargs):
    from concourse.bass_utils import run_bass_kernel_spmd

    nc = _get_nc()
    return run_bass_kernel_spmd(nc, in_maps, core_ids=list(range(NCORES)), **kwargs)


def postprocess(res):
    sim_out = np.empty((B, S, V), dtype=np.float32)
    sim_flat = sim_out.reshape(M, V)
    for c in range(NCORES):
        lo = c * VS
        hi = min(lo + VS, V)
        if hi <= lo:
            break
        sim_flat[:, lo:hi] = res.results[c]["sim"][:, :hi - lo]

    nn_idx = np.argmax(sim_flat, axis=1).astype(np.int32).reshape(B, S)
    return nn_idx, sim_out


def kernel(utterance, emb_table, data_grad):
    in_maps = prepare_in_maps(utterance, emb_table, data_grad)
    res = run_device(in_maps)
    return postprocess(res)
